# revision 8
# baseline (speedup 1.0000x reference)
"""Trainium2 Bass kernel for nn_Conv2D_80796924772741.

Depthwise (grouped, F=64) 3x3 valid conv over [F, 514, 514, 4] int8 with
per-channel int8 weights + int32 bias, followed by exact fixed-point requant
  res = (acc * 19920 + 2^21) >> 22 ;  out = clip(res - 5, -128, 127) int8
(reduced_mantissa 19920 = 1245 * 16 -> res = (acc*1245 + 2^17) >> 18).

Sharding: F=64 split across 8 NeuronCores (8 channels each).

Per-core pipeline, per (channel, H-window) group ([M<=124 rows, 2048 cols];
last 16 output rows packed 4-chunks-into-partitions as a [64, 512] strip):
 - PE:  conv via Toeplitz-band stationary matmuls over H-windows (3 W-taps,
        H-taps in the band diagonals, bias rides two all-ones rhs rows).
        PSUM = A = acc + b, exact fp32 (|A| <= 146161 < 2^24).
 - ACT: af32 = Copy(PSUM) -> SBUF fp32 (exact).
 - requant, one of two exact lanes (both verified bit-exact on HW for every
   possible A):
   lane G (GpSimd, 1 op, internally > fp32 precision):
        o8 = sat8(RNE(af * 1245/2^18 + (2^-19 - 5)))  == clip(res - 5)
   lane D (DVE, 6 ops, split A = 128*(h'+8) + (lo+1024)):
        hi16 = int16(af*2^-7 - 8.498046875)           = floor(A/128) - 8
        lo16 = int16(hi16*-128 + af)                  = A - 128*hi' (in [1024,1152))
        q16  = int16(lo16*9.7265625 - 9960.49609375)  = floor(lo*1245/128)
        qt16 = fp16(q16*2^-11 - 0.136474609375)       (exact in fp16)
        S16  = int16(hi16*0.60791015625 + qt16)       = res - 5 (RNE, no ties)
        o8   = int8(S16)                              (saturating == clip)
 - DMA y (scalar-engine HWDGE queue, delayed 2 groups to avoid HOL).
x is fed pre-converted to bf16 by the host with the two all-ones bias rows
baked in front of each window, so every x load is a single contiguous DMA.
"""

import numpy as np
import ml_dtypes

F_PER_CORE = 8
H_IN = 514
W_IN = 514
D = 4
H_OUT = 512
WD_OUT = 2048  # 512 * 4
FREE_IN = W_IN * D  # 2056
N_CHUNK = 512
N_CORES = 8

FULL_WINDOWS = [(0, 124), (124, 124), (248, 124), (372, 124)]
STRIP_R0 = 496
STRIP_M = 16  # output rows per chunk block
STRIP_KB = 20  # partitions per chunk block: 2 ones + 16+2 data rows

QS = 0.004749298095703125  # 1245 / 2^18
QD = (2.0 ** -19) - 5.0



def _build_lhsT(w_core: np.ndarray, b_core: np.ndarray) -> np.ndarray:
    """[128, 8*3*124] bf16 stationary: per (channel, w-tap) a Toeplitz band.

    Column block (f*3 + n)*124 : +124 holds T_n for channel f.
    T_n[2 + i + m, i] = w[f, m, n]  (rows 2.. are conv data partitions)
    T_0[0, i] = 8*floor(b/8) ; T_0[1, i] = b mod 8  (bias rows, multiplied
    by all-ones rhs partitions 0/1; both parts bf16-exact).
    """
    out = np.zeros((128, F_PER_CORE * 3 * 124), dtype=np.float32)
    for f in range(F_PER_CORE):
        b_f = int(b_core[f])
        bh = b_f >> 3
        bl = b_f - 8 * bh
        for n in range(3):
            base = (f * 3 + n) * 124
            if n == 0:
                out[0, base : base + 124] = float(8 * bh)
                out[1, base : base + 124] = float(bl)
            for m in range(3):
                wv = float(int(w_core[f, m, n, 0]))
                idx = np.arange(124)
                out[2 + idx + m, base + idx] = wv
    return out.astype(ml_dtypes.bfloat16)


def _build_lhsT2(w_core: np.ndarray, b_core: np.ndarray) -> np.ndarray:
    """[80, 8*3*64] bf16 strip stationaries, block-diagonal per chunk.

    Chunk block c occupies partitions 20c..20c+19 (2 ones rows + 18 data
    rows) and psum rows 16c..16c+15. Column block (f*3+n)*64 holds the
    tap-n stationary for channel f covering all 4 chunks.
    """
    out = np.zeros((80, F_PER_CORE * 3 * 64), dtype=np.float32)
    for f in range(F_PER_CORE):
        b_f = int(b_core[f])
        bh = b_f >> 3
        bl = b_f - 8 * bh
        for n in range(3):
            base = (f * 3 + n) * 64
            for c in range(4):
                col0 = base + 16 * c
                row0 = 20 * c
                if n == 0:
                    out[row0 + 0, col0 : col0 + 16] = float(8 * bh)
                    out[row0 + 1, col0 : col0 + 16] = float(bl)
                for m in range(3):
                    wv = float(int(w_core[f, m, n, 0]))
                    idx = np.arange(16)
                    out[row0 + 2 + idx + m, col0 + idx] = wv
    return out.astype(ml_dtypes.bfloat16)


_PROGRAM_CACHE = {}


def _build_program():
    import concourse.bass as bass
    import concourse.tile as tile
    from concourse import bacc, mybir

    nc = bacc.Bacc(
        "TRN2", target_bir_lowering=False, debug=False, num_devices=N_CORES
    )
    dt = mybir.dt
    Alu = mybir.AluOpType
    Act = mybir.ActivationFunctionType

    xa_d = nc.dram_tensor(
        "x_aug", [F_PER_CORE, 4, 128, FREE_IN], dt.bfloat16, kind="ExternalInput"
    ).ap()
    xs_d = nc.dram_tensor(
        "x_strip", [F_PER_CORE, 80, 520], dt.bfloat16, kind="ExternalInput"
    ).ap()
    lhsT_d = nc.dram_tensor(
        "lhsT", [128, F_PER_CORE * 3 * 124], dt.bfloat16, kind="ExternalInput"
    ).ap()
    lhsT2_d = nc.dram_tensor(
        "lhsT2", [80, F_PER_CORE * 3 * 64], dt.bfloat16, kind="ExternalInput"
    ).ap()
    y_d = nc.dram_tensor(
        "y", [F_PER_CORE, H_OUT, WD_OUT], dt.int8, kind="ExternalOutput"
    ).ap()

    with tile.TileContext(nc) as tc:
        with (
            tc.tile_pool(name="const", bufs=1) as const_pool,
            tc.tile_pool(name="xin", bufs=4) as x_pool,
            tc.tile_pool(name="psum", bufs=1, space="PSUM") as psum_pool,
            tc.tile_pool(name="af2", bufs=3) as af2_pool,
            tc.tile_pool(name="o82", bufs=3) as o82_pool,
            tc.tile_pool(name="afs", bufs=2) as afs_pool,
            tc.tile_pool(name="o8s", bufs=2) as o8s_pool,
        ):
            lhsT_t = const_pool.tile([128, F_PER_CORE * 3 * 124], dt.bfloat16)
            nc.sync.dma_start(lhsT_t[:], lhsT_d[:])
            lhsT2_t = const_pool.tile([80, F_PER_CORE * 3 * 64], dt.bfloat16)
            nc.sync.dma_start(lhsT2_t[:], lhsT2_d[:])
            warm = const_pool.tile([128, 8], dt.int32)
            nc.gpsimd.memset(warm[:], 0)
            nc.gpsimd.tensor_scalar(warm[:], warm[:], 1, 0, Alu.mult, Alu.add)

            pending_y = []
            n_copy = 0
            rot = 0

            def flush_y(limit):
                while len(pending_y) > limit:
                    emit = pending_y.pop(0)
                    emit()

            def emit_copy(dst, src):
                nonlocal n_copy
                if n_copy % 2 == 1:
                    nc.vector.tensor_copy(dst, src)
                else:
                    nc.scalar.activation(
                        dst, src, Act.Copy, bias=0.0, scale=1.0
                    )
                n_copy += 1

            def psum_tile():
                nonlocal rot
                t = psum_pool.tile([124, N_CHUNK], dt.float32, name=f"ps_{rot % 8}")
                rot += 1
                return t

            for f in range(F_PER_CORE):
                for pair in range(2):
                    w0 = 2 * pair
                    xts = []
                    for w in (w0, w0 + 1):
                        xt = x_pool.tile([128, FREE_IN], dt.bfloat16, name="xt")
                        nc.sync.dma_start(xt[:], xa_d[f, w])
                        xts.append(xt)
                    af2 = af2_pool.tile([124, 2 * WD_OUT], dt.float32)
                    for w in range(2):
                        for c in range(4):
                            ps = psum_tile()
                            for n in range(3):
                                base = (f * 3 + n) * 124
                                nc.tensor.matmul(
                                    ps[0:124, :],
                                    lhsT_t[0:128, base : base + 124],
                                    xts[w][0:128, c * N_CHUNK + 4 * n : c * N_CHUNK + 4 * n + N_CHUNK],
                                    start=(n == 0),
                                    stop=(n == 2),
                                    skip_group_check=True,
                                )
                            emit_copy(
                                af2[0:124, (4 * w + c) * N_CHUNK : (4 * w + c + 1) * N_CHUNK],
                                ps[0:124, :],
                            )
                    o82 = o82_pool.tile([124, 2 * WD_OUT], dt.int8)
                    nc.gpsimd.tensor_scalar(
                        o82[:], af2[:], QS, QD, Alu.mult, Alu.add
                    )
                    for w, rr in ((0, 124 * w0), (1, 124 * (w0 + 1))):
                        def emit_full(f=f, rr=rr, o82=o82, w=w):
                            nc.sync.dma_start(
                                y_d[f, rr : rr + 124, :],
                                o82[0:124, w * WD_OUT : (w + 1) * WD_OUT],
                            )
                        pending_y.append(emit_full)
                    flush_y(4)

            for f in range(F_PER_CORE):
                xt = x_pool.tile([128, FREE_IN], dt.bfloat16, name="xt")
                nc.sync.dma_start(xt[0:80, 0:520], xs_d[f])
                ps = psum_tile()
                for n in range(3):
                    base = (f * 3 + n) * 64
                    nc.tensor.matmul(
                        ps[0:64, 0:N_CHUNK],
                        lhsT2_t[0:80, base : base + 64],
                        xt[0:80, 4 * n : 4 * n + N_CHUNK],
                        start=(n == 0),
                        stop=(n == 2),
                        skip_group_check=True,
                    )
                afs = afs_pool.tile([124, N_CHUNK], dt.float32)
                nc.scalar.activation(
                    afs[0:64, :], ps[0:64, 0:N_CHUNK], Act.Copy, bias=0.0, scale=1.0
                )
                o8s = o8s_pool.tile([124, N_CHUNK], dt.int8)
                nc.gpsimd.tensor_scalar(
                    o8s[0:64, :], afs[0:64, :], QS, QD, Alu.mult, Alu.add
                )
                def emit_strip(f=f, o8s=o8s):
                    for c in range(4):
                        nc.scalar.dma_start(
                            y_d[f, STRIP_R0 : STRIP_R0 + STRIP_M,
                                c * N_CHUNK : (c + 1) * N_CHUNK],
                            o8s[16 * c : 16 * c + 16, 0:N_CHUNK],
                        )
                pending_y.append(emit_strip)
                flush_y(2)

            flush_y(0)

    nc.compile()
    return nc


def _make_in_maps(x: np.ndarray, w: np.ndarray, b: np.ndarray) -> list:
    bf16 = ml_dtypes.bfloat16
    in_maps = []
    for core in range(N_CORES):
        lo = core * F_PER_CORE
        hi = lo + F_PER_CORE
        x_bf = (
            np.ascontiguousarray(x[lo:hi])
            .reshape(F_PER_CORE, H_IN, FREE_IN)
            .astype(bf16)
        )
        x_aug = np.ones((F_PER_CORE, 4, 128, FREE_IN), dtype=bf16)
        for wi, (r0, m_r) in enumerate(FULL_WINDOWS):
            x_aug[:, wi, 2:128, :] = x_bf[:, r0 : r0 + 126, :]
        x_strip = np.ones((F_PER_CORE, 4, STRIP_KB, 520), dtype=bf16)
        for c in range(4):
            x_strip[:, c, 2:STRIP_KB, :] = x_bf[
                :, STRIP_R0 : STRIP_R0 + 18, c * WD_OUT // 4 : c * WD_OUT // 4 + 520
            ]
        in_maps.append(
            {
                "x_aug": x_aug,
                "x_strip": x_strip.reshape(F_PER_CORE, 80, 520),
                "lhsT": _build_lhsT(w[lo:hi], b[lo:hi]),
                "lhsT2": _build_lhsT2(w[lo:hi], b[lo:hi]),
            }
        )
    return in_maps


def kernel(x: np.ndarray, w: np.ndarray, b: np.ndarray) -> np.ndarray:
    """x: int8 [64, 514, 514, 4]; w: int8 [64, 3, 3, 1]; b: int32 [64].

    Returns int8 [64, 512, 512, 4].
    """
    from concourse.bass_utils import run_bass_kernel_spmd

    if "nc" not in _PROGRAM_CACHE:
        _PROGRAM_CACHE["nc"] = _build_program()
    nc = _PROGRAM_CACHE["nc"]

    F = x.shape[0]
    assert F == N_CORES * F_PER_CORE

    in_maps = _make_in_maps(x, w, b)
    res = run_bass_kernel_spmd(nc, in_maps, core_ids=list(range(N_CORES)))

    out = np.empty((F, H_OUT, 512, D), dtype=np.int8)
    for core in range(N_CORES):
        lo = core * F_PER_CORE
        y = res.results[core]["y"]  # [8, 512, 2048] int8
        out[lo : lo + F_PER_CORE] = y.reshape(F_PER_CORE, H_OUT, 512, D)
    return out


# revision 10
# speedup vs baseline: 1.1430x; 1.1430x over previous
"""Trainium2 Bass kernel for nn_Conv2D_80796924772741.

Depthwise (grouped, F=64) 3x3 valid conv over [F, 514, 514, 4] int8 with
per-channel int8 weights + int32 bias, followed by exact fixed-point requant
  res = (acc * 19920 + 2^21) >> 22 ;  out = clip(res - 5, -128, 127) int8
(reduced_mantissa 19920 = 1245 * 16 -> res = (acc*1245 + 2^17) >> 18).

Sharding: F=64 split across 8 NeuronCores (8 channels each).

Per-core pipeline, per (channel, H-window) group ([M<=124 rows, 2048 cols];
last 16 output rows packed 4-chunks-into-partitions as a [64, 512] strip):
 - PE:  conv via Toeplitz-band stationary matmuls over H-windows (3 W-taps,
        H-taps in the band diagonals, bias rides two all-ones rhs rows).
        PSUM = A = acc + b, exact fp32 (|A| <= 146161 < 2^24).
 - ACT: af32 = Copy(PSUM) -> SBUF fp32 (exact).
 - requant, one of two exact lanes (both verified bit-exact on HW for every
   possible A):
   lane G (GpSimd, 1 op, internally > fp32 precision):
        o8 = sat8(RNE(af * 1245/2^18 + (2^-19 - 5)))  == clip(res - 5)
   lane D (DVE, 6 ops, split A = 128*(h'+8) + (lo+1024)):
        hi16 = int16(af*2^-7 - 8.498046875)           = floor(A/128) - 8
        lo16 = int16(hi16*-128 + af)                  = A - 128*hi' (in [1024,1152))
        q16  = int16(lo16*9.7265625 - 9960.49609375)  = floor(lo*1245/128)
        qt16 = fp16(q16*2^-11 - 0.136474609375)       (exact in fp16)
        S16  = int16(hi16*0.60791015625 + qt16)       = res - 5 (RNE, no ties)
        o8   = int8(S16)                              (saturating == clip)
 - DMA y (scalar-engine HWDGE queue, delayed 2 groups to avoid HOL).
x is fed pre-converted to bf16 by the host with the two all-ones bias rows
baked in front of each window, so every x load is a single contiguous DMA.
"""

import numpy as np
import ml_dtypes

F_PER_CORE = 8
H_IN = 514
W_IN = 514
D = 4
H_OUT = 512
WD_OUT = 2048  # 512 * 4
FREE_IN = W_IN * D  # 2056
N_CHUNK = 512
N_CORES = 8

FULL_WINDOWS = [(0, 124), (124, 124), (248, 124), (372, 124)]
STRIP_R0 = 496
STRIP_M = 16  # output rows per chunk block
STRIP_KB = 20  # partitions per chunk block: 2 ones + 16+2 data rows

QS = 0.004749298095703125  # 1245 / 2^18
QD = (2.0 ** -19) - 5.0



def _build_lhsT(w_core: np.ndarray, b_core: np.ndarray) -> np.ndarray:
    """[128, 8*3*124] bf16 stationary: per (channel, w-tap) a Toeplitz band.

    Column block (f*3 + n)*124 : +124 holds T_n for channel f.
    T_n[2 + i + m, i] = w[f, m, n]  (rows 2.. are conv data partitions)
    T_0[0, i] = 8*floor(b/8) ; T_0[1, i] = b mod 8  (bias rows, multiplied
    by all-ones rhs partitions 0/1; both parts bf16-exact).
    """
    out = np.zeros((128, F_PER_CORE * 3 * 124), dtype=np.float32)
    for f in range(F_PER_CORE):
        b_f = int(b_core[f])
        bh = b_f >> 3
        bl = b_f - 8 * bh
        for n in range(3):
            base = (f * 3 + n) * 124
            if n == 0:
                out[0, base : base + 124] = float(8 * bh)
                out[1, base : base + 124] = float(bl)
            for m in range(3):
                wv = float(int(w_core[f, m, n, 0]))
                idx = np.arange(124)
                out[2 + idx + m, base + idx] = wv
    return out.astype(ml_dtypes.bfloat16)


def _build_lhsT2(w_core: np.ndarray, b_core: np.ndarray) -> np.ndarray:
    """[80, 8*3*64] bf16 strip stationaries, block-diagonal per chunk.

    Chunk block c occupies partitions 20c..20c+19 (2 ones rows + 18 data
    rows) and psum rows 16c..16c+15. Column block (f*3+n)*64 holds the
    tap-n stationary for channel f covering all 4 chunks.
    """
    out = np.zeros((80, F_PER_CORE * 3 * 64), dtype=np.float32)
    for f in range(F_PER_CORE):
        b_f = int(b_core[f])
        bh = b_f >> 3
        bl = b_f - 8 * bh
        for n in range(3):
            base = (f * 3 + n) * 64
            for c in range(4):
                col0 = base + 16 * c
                row0 = 20 * c
                if n == 0:
                    out[row0 + 0, col0 : col0 + 16] = float(8 * bh)
                    out[row0 + 1, col0 : col0 + 16] = float(bl)
                for m in range(3):
                    wv = float(int(w_core[f, m, n, 0]))
                    idx = np.arange(16)
                    out[row0 + 2 + idx + m, col0 + idx] = wv
    return out.astype(ml_dtypes.bfloat16)


_PROGRAM_CACHE = {}


def _build_program():
    import concourse.bass as bass
    import concourse.tile as tile
    from concourse import bacc, mybir

    nc = bacc.Bacc(
        "TRN2", target_bir_lowering=False, debug=False, num_devices=N_CORES
    )
    dt = mybir.dt
    Alu = mybir.AluOpType
    Act = mybir.ActivationFunctionType

    xa_d = nc.dram_tensor(
        "x_aug", [F_PER_CORE, 4, 128, FREE_IN], dt.bfloat16, kind="ExternalInput"
    ).ap()
    xs_d = nc.dram_tensor(
        "x_strip", [F_PER_CORE, 80, 520], dt.bfloat16, kind="ExternalInput"
    ).ap()
    lhsT_d = nc.dram_tensor(
        "lhsT", [128, F_PER_CORE * 3 * 124], dt.bfloat16, kind="ExternalInput"
    ).ap()
    lhsT2_d = nc.dram_tensor(
        "lhsT2", [80, F_PER_CORE * 3 * 64], dt.bfloat16, kind="ExternalInput"
    ).ap()
    y_d = nc.dram_tensor(
        "y", [F_PER_CORE, H_OUT, WD_OUT], dt.int8, kind="ExternalOutput"
    ).ap()

    groups = []
    for f in range(F_PER_CORE):
        for wi, (r0, m_r) in enumerate(FULL_WINDOWS):
            groups.append(("full", f, wi, r0, m_r))
        groups.append(("strip", f, 0, STRIP_R0, STRIP_M))

    with tile.TileContext(nc) as tc:
        with (
            tc.tile_pool(name="const", bufs=1) as const_pool,
            tc.tile_pool(name="xin", bufs=4) as x_pool,
            tc.tile_pool(name="psum", bufs=2, space="PSUM") as psum_pool,
            tc.tile_pool(name="af2", bufs=3) as af2_pool,
            tc.tile_pool(name="o82", bufs=3) as o82_pool,
            tc.tile_pool(name="afs", bufs=2) as afs_pool,
            tc.tile_pool(name="o8s", bufs=2) as o8s_pool,
        ):
            lhsT_t = const_pool.tile([128, F_PER_CORE * 3 * 124], dt.bfloat16)
            nc.sync.dma_start(lhsT_t[:], lhsT_d[:])
            lhsT2_t = const_pool.tile([80, F_PER_CORE * 3 * 64], dt.bfloat16)
            nc.sync.dma_start(lhsT2_t[:], lhsT2_d[:])
            warm = const_pool.tile([128, 8], dt.int32)
            nc.gpsimd.memset(warm[:], 0)
            nc.gpsimd.tensor_scalar(warm[:], warm[:], 1, 0, Alu.mult, Alu.add)

            pending_y = []
            c_full = 0

            def flush_y(limit):
                while len(pending_y) > limit:
                    emit = pending_y.pop(0)
                    emit()

            af2 = None
            for kind, f, wi, r0, m_r in groups:
                if kind == "full":
                    xt = x_pool.tile([128, FREE_IN], dt.bfloat16, name="xt")
                    nc.sync.dma_start(xt[:], xa_d[f, wi])
                    ps = psum_pool.tile([124, WD_OUT], dt.float32, name="ps")
                    for n in range(3):
                        base = (f * 3 + n) * 124
                        for c in range(4):
                            nc.tensor.matmul(
                                ps[0:124, c * N_CHUNK : (c + 1) * N_CHUNK],
                                lhsT_t[0:128, base : base + 124],
                                xt[0:128, c * N_CHUNK + 4 * n : c * N_CHUNK + 4 * n + N_CHUNK],
                                start=(n == 0),
                                stop=(n == 2),
                                skip_group_check=True,
                            )
                    half = wi % 2
                    if half == 0:
                        af2 = af2_pool.tile([124, 2 * WD_OUT], dt.float32, name="af2")
                    dst = af2[0:124, half * WD_OUT : (half + 1) * WD_OUT]
                    if c_full % 2 == 1:
                        nc.vector.tensor_copy(dst, ps[0:124, :])
                    else:
                        nc.scalar.activation(
                            dst, ps[0:124, :], Act.Copy, bias=0.0, scale=1.0
                        )
                    c_full += 1
                    if half == 1:
                        o82 = o82_pool.tile([124, 2 * WD_OUT], dt.int8, name="o82")
                        nc.gpsimd.tensor_scalar(
                            o82[:], af2[:], QS, QD, Alu.mult, Alu.add
                        )
                        for h, rr in ((0, r0 - 124), (1, r0)):
                            def emit_full(f=f, rr=rr, o82=o82, h=h):
                                nc.scalar.dma_start(
                                    y_d[f, rr : rr + 124, :],
                                    o82[0:124, h * WD_OUT : (h + 1) * WD_OUT],
                                )
                            pending_y.append(emit_full)
                else:
                    xt = x_pool.tile([128, FREE_IN], dt.bfloat16, name="xt")
                    nc.sync.dma_start(xt[0:80, 0:520], xs_d[f])
                    ps = psum_pool.tile([124, WD_OUT], dt.float32, name="ps")
                    for n in range(3):
                        base = (f * 3 + n) * 64
                        nc.tensor.matmul(
                            ps[0:64, 0:N_CHUNK],
                            lhsT2_t[0:80, base : base + 64],
                            xt[0:80, 4 * n : 4 * n + N_CHUNK],
                            start=(n == 0),
                            stop=(n == 2),
                            skip_group_check=True,
                        )
                    afs = afs_pool.tile([124, N_CHUNK], dt.float32)
                    nc.scalar.activation(
                        afs[0:64, :], ps[0:64, 0:N_CHUNK], Act.Copy,
                        bias=0.0, scale=1.0,
                    )
                    o8s = o8s_pool.tile([124, N_CHUNK], dt.int8)
                    nc.gpsimd.tensor_scalar(
                        o8s[0:64, :], afs[0:64, :], QS, QD, Alu.mult, Alu.add
                    )
                    def emit_strip(f=f, r0=r0, o8s=o8s):
                        for c in range(4):
                            nc.scalar.dma_start(
                                y_d[f, r0 : r0 + STRIP_M,
                                    c * N_CHUNK : (c + 1) * N_CHUNK],
                                o8s[16 * c : 16 * c + 16, 0:N_CHUNK],
                            )
                    pending_y.append(emit_strip)
                flush_y(3)

            flush_y(0)

    nc.compile()
    return nc


def _make_in_maps(x: np.ndarray, w: np.ndarray, b: np.ndarray) -> list:
    bf16 = ml_dtypes.bfloat16
    in_maps = []
    for core in range(N_CORES):
        lo = core * F_PER_CORE
        hi = lo + F_PER_CORE
        x_bf = (
            np.ascontiguousarray(x[lo:hi])
            .reshape(F_PER_CORE, H_IN, FREE_IN)
            .astype(bf16)
        )
        x_aug = np.ones((F_PER_CORE, 4, 128, FREE_IN), dtype=bf16)
        for wi, (r0, m_r) in enumerate(FULL_WINDOWS):
            x_aug[:, wi, 2:128, :] = x_bf[:, r0 : r0 + 126, :]
        x_strip = np.ones((F_PER_CORE, 4, STRIP_KB, 520), dtype=bf16)
        for c in range(4):
            x_strip[:, c, 2:STRIP_KB, :] = x_bf[
                :, STRIP_R0 : STRIP_R0 + 18, c * WD_OUT // 4 : c * WD_OUT // 4 + 520
            ]
        in_maps.append(
            {
                "x_aug": x_aug,
                "x_strip": x_strip.reshape(F_PER_CORE, 80, 520),
                "lhsT": _build_lhsT(w[lo:hi], b[lo:hi]),
                "lhsT2": _build_lhsT2(w[lo:hi], b[lo:hi]),
            }
        )
    return in_maps


def kernel(x: np.ndarray, w: np.ndarray, b: np.ndarray) -> np.ndarray:
    """x: int8 [64, 514, 514, 4]; w: int8 [64, 3, 3, 1]; b: int32 [64].

    Returns int8 [64, 512, 512, 4].
    """
    from concourse.bass_utils import run_bass_kernel_spmd

    if "nc" not in _PROGRAM_CACHE:
        _PROGRAM_CACHE["nc"] = _build_program()
    nc = _PROGRAM_CACHE["nc"]

    F = x.shape[0]
    assert F == N_CORES * F_PER_CORE

    in_maps = _make_in_maps(x, w, b)
    res = run_bass_kernel_spmd(nc, in_maps, core_ids=list(range(N_CORES)))

    out = np.empty((F, H_OUT, 512, D), dtype=np.int8)
    for core in range(N_CORES):
        lo = core * F_PER_CORE
        y = res.results[core]["y"]  # [8, 512, 2048] int8
        out[lo : lo + F_PER_CORE] = y.reshape(F_PER_CORE, H_OUT, 512, D)
    return out


# revision 12
# speedup vs baseline: 1.1795x; 1.0320x over previous
"""Trainium2 Bass kernel for nn_Conv2D_80796924772741.

Depthwise (grouped, F=64) 3x3 valid conv over [F, 514, 514, 4] int8 with
per-channel int8 weights + int32 bias, followed by exact fixed-point requant
  res = (acc * 19920 + 2^21) >> 22 ;  out = clip(res - 5, -128, 127) int8
(reduced_mantissa 19920 = 1245 * 16 -> res = (acc*1245 + 2^17) >> 18).

Sharding: F=64 split across 8 NeuronCores (8 channels each).

Per-core pipeline, per (channel, H-window) group ([M<=124 rows, 2048 cols];
last 16 output rows packed 4-chunks-into-partitions as a [64, 512] strip):
 - PE:  conv via Toeplitz-band stationary matmuls over H-windows (3 W-taps,
        H-taps in the band diagonals, bias rides two all-ones rhs rows).
        PSUM = A = acc + b, exact fp32 (|A| <= 146161 < 2^24).
 - ACT: af32 = Copy(PSUM) -> SBUF fp32 (exact).
 - requant, one of two exact lanes (both verified bit-exact on HW for every
   possible A):
   lane G (GpSimd, 1 op, internally > fp32 precision):
        o8 = sat8(RNE(af * 1245/2^18 + (2^-19 - 5)))  == clip(res - 5)
   lane D (DVE, 6 ops, split A = 128*(h'+8) + (lo+1024)):
        hi16 = int16(af*2^-7 - 8.498046875)           = floor(A/128) - 8
        lo16 = int16(hi16*-128 + af)                  = A - 128*hi' (in [1024,1152))
        q16  = int16(lo16*9.7265625 - 9960.49609375)  = floor(lo*1245/128)
        qt16 = fp16(q16*2^-11 - 0.136474609375)       (exact in fp16)
        S16  = int16(hi16*0.60791015625 + qt16)       = res - 5 (RNE, no ties)
        o8   = int8(S16)                              (saturating == clip)
 - DMA y (scalar-engine HWDGE queue, delayed 2 groups to avoid HOL).
x is fed pre-converted to bf16 by the host with the two all-ones bias rows
baked in front of each window, so every x load is a single contiguous DMA.
"""

import numpy as np
import ml_dtypes

F_PER_CORE = 8
H_IN = 514
W_IN = 514
D = 4
H_OUT = 512
WD_OUT = 2048  # 512 * 4
FREE_IN = W_IN * D  # 2056
N_CHUNK = 512
N_CORES = 8

FULL_WINDOWS = [(0, 124), (124, 124), (248, 124), (372, 124)]
STRIP_R0 = 496
STRIP_M = 16  # output rows per chunk block
STRIP_KB = 20  # partitions per chunk block: 2 ones + 16+2 data rows

QS = 0.004749298095703125  # 1245 / 2^18
QD = (2.0 ** -19) - 5.0



def _build_lhsT(w_core: np.ndarray, b_core: np.ndarray) -> np.ndarray:
    """[128, 8*3*124] bf16 stationary: per (channel, w-tap) a Toeplitz band.

    Column block (f*3 + n)*124 : +124 holds T_n for channel f.
    T_n[2 + i + m, i] = w[f, m, n]  (rows 2.. are conv data partitions)
    T_0[0, i] = 8*floor(b/8) ; T_0[1, i] = b mod 8  (bias rows, multiplied
    by all-ones rhs partitions 0/1; both parts bf16-exact).
    """
    out = np.zeros((128, F_PER_CORE * 3 * 124), dtype=np.float32)
    for f in range(F_PER_CORE):
        b_f = int(b_core[f])
        bh = b_f >> 3
        bl = b_f - 8 * bh
        for n in range(3):
            base = (f * 3 + n) * 124
            if n == 0:
                out[0, base : base + 124] = float(8 * bh)
                out[1, base : base + 124] = float(bl)
            for m in range(3):
                wv = float(int(w_core[f, m, n, 0]))
                idx = np.arange(124)
                out[2 + idx + m, base + idx] = wv
    return out.astype(ml_dtypes.bfloat16)


def _build_lhsT2(w_core: np.ndarray, b_core: np.ndarray) -> np.ndarray:
    """[80, 8*3*64] bf16 strip stationaries, block-diagonal per chunk.

    Chunk block c occupies partitions 20c..20c+19 (2 ones rows + 18 data
    rows) and psum rows 16c..16c+15. Column block (f*3+n)*64 holds the
    tap-n stationary for channel f covering all 4 chunks.
    """
    out = np.zeros((80, F_PER_CORE * 3 * 64), dtype=np.float32)
    for f in range(F_PER_CORE):
        b_f = int(b_core[f])
        bh = b_f >> 3
        bl = b_f - 8 * bh
        for n in range(3):
            base = (f * 3 + n) * 64
            for c in range(4):
                col0 = base + 16 * c
                row0 = 20 * c
                if n == 0:
                    out[row0 + 0, col0 : col0 + 16] = float(8 * bh)
                    out[row0 + 1, col0 : col0 + 16] = float(bl)
                for m in range(3):
                    wv = float(int(w_core[f, m, n, 0]))
                    idx = np.arange(16)
                    out[row0 + 2 + idx + m, col0 + idx] = wv
    return out.astype(ml_dtypes.bfloat16)


_PROGRAM_CACHE = {}


def _build_program():
    import concourse.bass as bass
    import concourse.tile as tile
    from concourse import bacc, mybir

    nc = bacc.Bacc(
        "TRN2", target_bir_lowering=False, debug=False, num_devices=N_CORES
    )
    dt = mybir.dt
    Alu = mybir.AluOpType
    Act = mybir.ActivationFunctionType

    xa_d = nc.dram_tensor(
        "x_pair", [F_PER_CORE, 2, 128, 2 * FREE_IN], dt.bfloat16, kind="ExternalInput"
    ).ap()
    xs_d = nc.dram_tensor(
        "x_strip", [F_PER_CORE, 80, 520], dt.bfloat16, kind="ExternalInput"
    ).ap()
    lhsT_d = nc.dram_tensor(
        "lhsT", [128, F_PER_CORE * 3 * 124], dt.bfloat16, kind="ExternalInput"
    ).ap()
    lhsT2_d = nc.dram_tensor(
        "lhsT2", [80, F_PER_CORE * 3 * 64], dt.bfloat16, kind="ExternalInput"
    ).ap()
    y_d = nc.dram_tensor(
        "y", [F_PER_CORE, H_OUT, WD_OUT], dt.int8, kind="ExternalOutput"
    ).ap()

    with tile.TileContext(nc) as tc:
        with (
            tc.tile_pool(name="const", bufs=1) as const_pool,
            tc.tile_pool(name="xin", bufs=4) as x_pool,
            tc.tile_pool(name="psum", bufs=2, space="PSUM") as psum_pool,
            tc.tile_pool(name="af2", bufs=4) as af2_pool,
            tc.tile_pool(name="o82", bufs=4) as o82_pool,
            tc.tile_pool(name="afs", bufs=2) as afs_pool,
            tc.tile_pool(name="o8s", bufs=2) as o8s_pool,
        ):
            lhsT_t = const_pool.tile([128, F_PER_CORE * 3 * 124], dt.bfloat16)
            nc.sync.dma_start(lhsT_t[:], lhsT_d[:])
            lhsT2_t = const_pool.tile([80, F_PER_CORE * 3 * 64], dt.bfloat16)
            nc.sync.dma_start(lhsT2_t[:], lhsT2_d[:])
            warm = const_pool.tile([128, 8], dt.int32)
            nc.gpsimd.memset(warm[:], 0)
            nc.gpsimd.tensor_scalar(warm[:], warm[:], 1, 0, Alu.mult, Alu.add)

            pending_y = []
            n_copy = 0

            def flush_y(limit):
                while len(pending_y) > limit:
                    emit = pending_y.pop(0)
                    emit()

            def emit_copy(dst, src):
                nonlocal n_copy
                if n_copy % 2 == 1:
                    nc.vector.tensor_copy(dst, src)
                else:
                    nc.scalar.activation(
                        dst, src, Act.Copy, bias=0.0, scale=1.0
                    )
                n_copy += 1

            def do_pair(f, pair, last=False):
                w0 = 2 * pair
                xt = x_pool.tile([128, 2 * FREE_IN], dt.bfloat16, name="xt")
                nc.sync.dma_start(xt[:], xa_d[f, pair])
                af2 = af2_pool.tile([124, 2 * WD_OUT], dt.float32, name="af2")
                o82 = o82_pool.tile([124, 2 * WD_OUT], dt.int8, name="o82")
                for w in range(2):
                    ps = psum_pool.tile([124, WD_OUT], dt.float32, name="ps")
                    for n in range(3):
                        base = (f * 3 + n) * 124
                        for c in range(4):
                            off = w * FREE_IN + c * N_CHUNK + 4 * n
                            nc.tensor.matmul(
                                ps[0:124, c * N_CHUNK : (c + 1) * N_CHUNK],
                                lhsT_t[0:128, base : base + 124],
                                xt[0:128, off : off + N_CHUNK],
                                start=(n == 0),
                                stop=(n == 2),
                                skip_group_check=True,
                            )
                    emit_copy(
                        af2[0:124, w * WD_OUT : (w + 1) * WD_OUT], ps[0:124, :]
                    )
                    if last:
                        nc.gpsimd.tensor_scalar(
                            o82[0:124, w * WD_OUT : (w + 1) * WD_OUT],
                            af2[0:124, w * WD_OUT : (w + 1) * WD_OUT],
                            QS, QD, Alu.mult, Alu.add,
                        )
                if not last:
                    nc.gpsimd.tensor_scalar(
                        o82[:], af2[:], QS, QD, Alu.mult, Alu.add
                    )
                for w, rr in ((0, 124 * w0), (1, 124 * (w0 + 1))):
                    def emit_full(f=f, rr=rr, o82=o82, w=w):
                        nc.sync.dma_start(
                            y_d[f, rr : rr + 124, :],
                            o82[0:124, w * WD_OUT : (w + 1) * WD_OUT],
                        )
                    pending_y.append(emit_full)

            def do_strip(f):
                xt = x_pool.tile([128, 2 * FREE_IN], dt.bfloat16, name="xt")
                nc.sync.dma_start(xt[0:80, 0:520], xs_d[f])
                ps = psum_pool.tile([124, WD_OUT], dt.float32, name="ps")
                for n in range(3):
                    base = (f * 3 + n) * 64
                    nc.tensor.matmul(
                        ps[0:64, 0:N_CHUNK],
                        lhsT2_t[0:80, base : base + 64],
                        xt[0:80, 4 * n : 4 * n + N_CHUNK],
                        start=(n == 0),
                        stop=(n == 2),
                        skip_group_check=True,
                    )
                afs = afs_pool.tile([124, N_CHUNK], dt.float32)
                nc.scalar.activation(
                    afs[0:64, :], ps[0:64, 0:N_CHUNK], Act.Copy, bias=0.0, scale=1.0
                )
                o8s = o8s_pool.tile([124, N_CHUNK], dt.int8)
                nc.gpsimd.tensor_scalar(
                    o8s[0:64, :], afs[0:64, :], QS, QD, Alu.mult, Alu.add
                )
                def emit_strip(f=f, o8s=o8s):
                    for c in range(4):
                        nc.scalar.dma_start(
                            y_d[f, STRIP_R0 : STRIP_R0 + STRIP_M,
                                c * N_CHUNK : (c + 1) * N_CHUNK],
                            o8s[16 * c : 16 * c + 16, 0:N_CHUNK],
                        )
                pending_y.append(emit_strip)

            for f in range(F_PER_CORE):
                last_f = f == F_PER_CORE - 1
                do_pair(f, 0)
                flush_y(4)
                do_strip(f)
                flush_y(4)
                do_pair(f, 1, last=last_f)
                flush_y(4 if not last_f else 0)

            flush_y(0)

    nc.compile()
    return nc


def _make_in_maps(x: np.ndarray, w: np.ndarray, b: np.ndarray) -> list:
    bf16 = ml_dtypes.bfloat16
    in_maps = []
    for core in range(N_CORES):
        lo = core * F_PER_CORE
        hi = lo + F_PER_CORE
        x_bf = (
            np.ascontiguousarray(x[lo:hi])
            .reshape(F_PER_CORE, H_IN, FREE_IN)
            .astype(bf16)
        )
        x_pair = np.ones((F_PER_CORE, 2, 128, 2 * FREE_IN), dtype=bf16)
        for wi, (r0, m_r) in enumerate(FULL_WINDOWS):
            x_pair[:, wi // 2, 2:128, (wi % 2) * FREE_IN : (wi % 2 + 1) * FREE_IN] = (
                x_bf[:, r0 : r0 + 126, :]
            )
        x_strip = np.ones((F_PER_CORE, 4, STRIP_KB, 520), dtype=bf16)
        for c in range(4):
            x_strip[:, c, 2:STRIP_KB, :] = x_bf[
                :, STRIP_R0 : STRIP_R0 + 18, c * N_CHUNK : c * N_CHUNK + 520
            ]
        in_maps.append(
            {
                "x_pair": x_pair,
                "x_strip": x_strip.reshape(F_PER_CORE, 80, 520),
                "lhsT": _build_lhsT(w[lo:hi], b[lo:hi]),
                "lhsT2": _build_lhsT2(w[lo:hi], b[lo:hi]),
            }
        )
    return in_maps


def kernel(x: np.ndarray, w: np.ndarray, b: np.ndarray) -> np.ndarray:
    """x: int8 [64, 514, 514, 4]; w: int8 [64, 3, 3, 1]; b: int32 [64].

    Returns int8 [64, 512, 512, 4].
    """
    from concourse.bass_utils import run_bass_kernel_spmd

    if "nc" not in _PROGRAM_CACHE:
        _PROGRAM_CACHE["nc"] = _build_program()
    nc = _PROGRAM_CACHE["nc"]

    F = x.shape[0]
    assert F == N_CORES * F_PER_CORE

    in_maps = _make_in_maps(x, w, b)
    res = run_bass_kernel_spmd(nc, in_maps, core_ids=list(range(N_CORES)))

    out = np.empty((F, H_OUT, 512, D), dtype=np.int8)
    for core in range(N_CORES):
        lo = core * F_PER_CORE
        y = res.results[core]["y"]  # [8, 512, 2048] int8
        out[lo : lo + F_PER_CORE] = y.reshape(F_PER_CORE, H_OUT, 512, D)
    return out


# revision 13
# speedup vs baseline: 1.1990x; 1.0165x over previous
"""Trainium2 Bass kernel for nn_Conv2D_80796924772741.

Depthwise (grouped, F=64) 3x3 valid conv over [F, 514, 514, 4] int8 with
per-channel int8 weights + int32 bias, followed by exact fixed-point requant
  res = (acc * 19920 + 2^21) >> 22 ;  out = clip(res - 5, -128, 127) int8
(reduced_mantissa 19920 = 1245 * 16 -> res = (acc*1245 + 2^17) >> 18).

Sharding: F=64 split across 8 NeuronCores (8 channels each).

Per-core pipeline, per (channel, H-window) group ([M<=124 rows, 2048 cols];
last 16 output rows packed 4-chunks-into-partitions as a [64, 512] strip):
 - PE:  conv via Toeplitz-band stationary matmuls over H-windows (3 W-taps,
        H-taps in the band diagonals, bias rides two all-ones rhs rows).
        PSUM = A = acc + b, exact fp32 (|A| <= 146161 < 2^24).
 - ACT: af32 = Copy(PSUM) -> SBUF fp32 (exact).
 - requant, one of two exact lanes (both verified bit-exact on HW for every
   possible A):
   lane G (GpSimd, 1 op, internally > fp32 precision):
        o8 = sat8(RNE(af * 1245/2^18 + (2^-19 - 5)))  == clip(res - 5)
   lane D (DVE, 6 ops, split A = 128*(h'+8) + (lo+1024)):
        hi16 = int16(af*2^-7 - 8.498046875)           = floor(A/128) - 8
        lo16 = int16(hi16*-128 + af)                  = A - 128*hi' (in [1024,1152))
        q16  = int16(lo16*9.7265625 - 9960.49609375)  = floor(lo*1245/128)
        qt16 = fp16(q16*2^-11 - 0.136474609375)       (exact in fp16)
        S16  = int16(hi16*0.60791015625 + qt16)       = res - 5 (RNE, no ties)
        o8   = int8(S16)                              (saturating == clip)
 - DMA y (scalar-engine HWDGE queue, delayed 2 groups to avoid HOL).
x is fed pre-converted to bf16 by the host with the two all-ones bias rows
baked in front of each window, so every x load is a single contiguous DMA.
"""

import numpy as np
import ml_dtypes

F_PER_CORE = 8
H_IN = 514
W_IN = 514
D = 4
H_OUT = 512
WD_OUT = 2048  # 512 * 4
FREE_IN = W_IN * D  # 2056
N_CHUNK = 512
N_CORES = 8

FULL_WINDOWS = [(0, 124), (124, 124), (248, 124), (372, 124)]
STRIP_R0 = 496
STRIP_M = 16  # output rows per chunk block
STRIP_KB = 20  # partitions per chunk block: 2 ones + 16+2 data rows

QS = 0.004749298095703125  # 1245 / 2^18
QD = (2.0 ** -19) - 5.0



def _build_lhsT(w_core: np.ndarray, b_core: np.ndarray) -> np.ndarray:
    """[128, 8*3*124] bf16 stationary: per (channel, w-tap) a Toeplitz band.

    Column block (f*3 + n)*124 : +124 holds T_n for channel f.
    T_n[2 + i + m, i] = w[f, m, n]  (rows 2.. are conv data partitions)
    T_0[0, i] = 8*floor(b/8) ; T_0[1, i] = b mod 8  (bias rows, multiplied
    by all-ones rhs partitions 0/1; both parts bf16-exact).
    """
    out = np.zeros((128, F_PER_CORE * 3 * 124), dtype=np.float32)
    for f in range(F_PER_CORE):
        b_f = int(b_core[f])
        bh = b_f >> 3
        bl = b_f - 8 * bh
        for n in range(3):
            base = (f * 3 + n) * 124
            if n == 0:
                out[0, base : base + 124] = float(8 * bh)
                out[1, base : base + 124] = float(bl)
            for m in range(3):
                wv = float(int(w_core[f, m, n, 0]))
                idx = np.arange(124)
                out[2 + idx + m, base + idx] = wv
    return out.astype(ml_dtypes.bfloat16)


def _build_lhsT2(w_core: np.ndarray, b_core: np.ndarray) -> np.ndarray:
    """[80, 8*3*64] bf16 strip stationaries, block-diagonal per chunk.

    Chunk block c occupies partitions 20c..20c+19 (2 ones rows + 18 data
    rows) and psum rows 16c..16c+15. Column block (f*3+n)*64 holds the
    tap-n stationary for channel f covering all 4 chunks.
    """
    out = np.zeros((80, F_PER_CORE * 3 * 64), dtype=np.float32)
    for f in range(F_PER_CORE):
        b_f = int(b_core[f])
        bh = b_f >> 3
        bl = b_f - 8 * bh
        for n in range(3):
            base = (f * 3 + n) * 64
            for c in range(4):
                col0 = base + 16 * c
                row0 = 20 * c
                if n == 0:
                    out[row0 + 0, col0 : col0 + 16] = float(8 * bh)
                    out[row0 + 1, col0 : col0 + 16] = float(bl)
                for m in range(3):
                    wv = float(int(w_core[f, m, n, 0]))
                    idx = np.arange(16)
                    out[row0 + 2 + idx + m, col0 + idx] = wv
    return out.astype(ml_dtypes.bfloat16)


_PROGRAM_CACHE = {}


def _build_program():
    import concourse.bass as bass
    import concourse.tile as tile
    from concourse import bacc, mybir

    nc = bacc.Bacc(
        "TRN2", target_bir_lowering=False, debug=False, num_devices=N_CORES
    )
    dt = mybir.dt
    Alu = mybir.AluOpType
    Act = mybir.ActivationFunctionType

    xa_d = nc.dram_tensor(
        "x_pair", [F_PER_CORE, 2, 128, 2 * FREE_IN], dt.bfloat16, kind="ExternalInput"
    ).ap()
    xs_d = nc.dram_tensor(
        "x_strip", [F_PER_CORE, 80, 520], dt.bfloat16, kind="ExternalInput"
    ).ap()
    lhsT_d = nc.dram_tensor(
        "lhsT", [128, F_PER_CORE * 3 * 124], dt.bfloat16, kind="ExternalInput"
    ).ap()
    lhsT2_d = nc.dram_tensor(
        "lhsT2", [80, F_PER_CORE * 3 * 64], dt.bfloat16, kind="ExternalInput"
    ).ap()
    # y rows padded 2048 -> 2304 bytes: a row stride of exactly 2048B makes
    # every store descriptor hash to the same 4 of 16 DMA engines.
    y_d = nc.dram_tensor(
        "y", [F_PER_CORE, H_OUT, WD_OUT + 256], dt.int8, kind="ExternalOutput"
    ).ap()

    with tile.TileContext(nc) as tc:
        with (
            tc.tile_pool(name="const", bufs=1) as const_pool,
            tc.tile_pool(name="xin", bufs=4) as x_pool,
            tc.tile_pool(name="psum", bufs=2, space="PSUM") as psum_pool,
            tc.tile_pool(name="af2", bufs=4) as af2_pool,
            tc.tile_pool(name="o82", bufs=4) as o82_pool,
            tc.tile_pool(name="afs", bufs=2) as afs_pool,
            tc.tile_pool(name="o8s", bufs=2) as o8s_pool,
        ):
            lhsT_t = const_pool.tile([128, F_PER_CORE * 3 * 124], dt.bfloat16)
            nc.sync.dma_start(lhsT_t[:], lhsT_d[:])
            lhsT2_t = const_pool.tile([80, F_PER_CORE * 3 * 64], dt.bfloat16)
            nc.sync.dma_start(lhsT2_t[:], lhsT2_d[:])
            warm = const_pool.tile([128, 8], dt.int32)
            nc.gpsimd.memset(warm[:], 0)
            nc.gpsimd.tensor_scalar(warm[:], warm[:], 1, 0, Alu.mult, Alu.add)

            pending_y = []
            n_copy = 0

            def flush_y(limit):
                while len(pending_y) > limit:
                    emit = pending_y.pop(0)
                    emit()

            def emit_copy(dst, src):
                nonlocal n_copy
                if n_copy % 2 == 1:
                    nc.vector.tensor_copy(dst, src)
                else:
                    nc.scalar.activation(
                        dst, src, Act.Copy, bias=0.0, scale=1.0
                    )
                n_copy += 1

            def do_pair(f, pair, last=False):
                w0 = 2 * pair
                xt = x_pool.tile([128, 2 * FREE_IN], dt.bfloat16, name="xt")
                nc.sync.dma_start(xt[:], xa_d[f, pair])
                af2 = af2_pool.tile([124, 2 * WD_OUT], dt.float32, name="af2")
                o82 = o82_pool.tile([124, 2 * WD_OUT], dt.int8, name="o82")
                for w in range(2):
                    ps = psum_pool.tile([124, WD_OUT], dt.float32, name="ps")
                    for n in range(3):
                        base = (f * 3 + n) * 124
                        for c in range(4):
                            off = w * FREE_IN + c * N_CHUNK + 4 * n
                            nc.tensor.matmul(
                                ps[0:124, c * N_CHUNK : (c + 1) * N_CHUNK],
                                lhsT_t[0:128, base : base + 124],
                                xt[0:128, off : off + N_CHUNK],
                                start=(n == 0),
                                stop=(n == 2),
                                skip_group_check=True,
                            )
                    emit_copy(
                        af2[0:124, w * WD_OUT : (w + 1) * WD_OUT], ps[0:124, :]
                    )
                    if last:
                        nc.gpsimd.tensor_scalar(
                            o82[0:124, w * WD_OUT : (w + 1) * WD_OUT],
                            af2[0:124, w * WD_OUT : (w + 1) * WD_OUT],
                            QS, QD, Alu.mult, Alu.add,
                        )
                if not last:
                    nc.gpsimd.tensor_scalar(
                        o82[:], af2[:], QS, QD, Alu.mult, Alu.add
                    )
                for w, rr in ((0, 124 * w0), (1, 124 * (w0 + 1))):
                    def emit_full(f=f, rr=rr, o82=o82, w=w):
                        nc.sync.dma_start(
                            y_d[f, rr : rr + 124, 0:WD_OUT],
                            o82[0:124, w * WD_OUT : (w + 1) * WD_OUT],
                        )
                    pending_y.append(emit_full)

            def do_strip(f):
                xt = x_pool.tile([128, 2 * FREE_IN], dt.bfloat16, name="xt")
                nc.sync.dma_start(xt[0:80, 0:520], xs_d[f])
                ps = psum_pool.tile([124, WD_OUT], dt.float32, name="ps")
                for n in range(3):
                    base = (f * 3 + n) * 64
                    nc.tensor.matmul(
                        ps[0:64, 0:N_CHUNK],
                        lhsT2_t[0:80, base : base + 64],
                        xt[0:80, 4 * n : 4 * n + N_CHUNK],
                        start=(n == 0),
                        stop=(n == 2),
                        skip_group_check=True,
                    )
                afs = afs_pool.tile([124, N_CHUNK], dt.float32)
                nc.scalar.activation(
                    afs[0:64, :], ps[0:64, 0:N_CHUNK], Act.Copy, bias=0.0, scale=1.0
                )
                o8s = o8s_pool.tile([124, N_CHUNK], dt.int8)
                nc.gpsimd.tensor_scalar(
                    o8s[0:64, :], afs[0:64, :], QS, QD, Alu.mult, Alu.add
                )
                def emit_strip(f=f, o8s=o8s):
                    for c in range(4):
                        nc.scalar.dma_start(
                            y_d[f, STRIP_R0 : STRIP_R0 + STRIP_M,
                                c * N_CHUNK : (c + 1) * N_CHUNK],
                            o8s[16 * c : 16 * c + 16, 0:N_CHUNK],
                        )
                pending_y.append(emit_strip)

            for f in range(F_PER_CORE):
                last_f = f == F_PER_CORE - 1
                do_pair(f, 0)
                flush_y(4)
                do_strip(f)
                flush_y(4)
                do_pair(f, 1, last=last_f)
                flush_y(4 if not last_f else 0)

            flush_y(0)

    nc.compile()
    return nc


def _make_in_maps(x: np.ndarray, w: np.ndarray, b: np.ndarray) -> list:
    bf16 = ml_dtypes.bfloat16
    in_maps = []
    for core in range(N_CORES):
        lo = core * F_PER_CORE
        hi = lo + F_PER_CORE
        x_bf = (
            np.ascontiguousarray(x[lo:hi])
            .reshape(F_PER_CORE, H_IN, FREE_IN)
            .astype(bf16)
        )
        x_pair = np.ones((F_PER_CORE, 2, 128, 2 * FREE_IN), dtype=bf16)
        for wi, (r0, m_r) in enumerate(FULL_WINDOWS):
            x_pair[:, wi // 2, 2:128, (wi % 2) * FREE_IN : (wi % 2 + 1) * FREE_IN] = (
                x_bf[:, r0 : r0 + 126, :]
            )
        x_strip = np.ones((F_PER_CORE, 4, STRIP_KB, 520), dtype=bf16)
        for c in range(4):
            x_strip[:, c, 2:STRIP_KB, :] = x_bf[
                :, STRIP_R0 : STRIP_R0 + 18, c * N_CHUNK : c * N_CHUNK + 520
            ]
        in_maps.append(
            {
                "x_pair": x_pair,
                "x_strip": x_strip.reshape(F_PER_CORE, 80, 520),
                "lhsT": _build_lhsT(w[lo:hi], b[lo:hi]),
                "lhsT2": _build_lhsT2(w[lo:hi], b[lo:hi]),
            }
        )
    return in_maps


def kernel(x: np.ndarray, w: np.ndarray, b: np.ndarray) -> np.ndarray:
    """x: int8 [64, 514, 514, 4]; w: int8 [64, 3, 3, 1]; b: int32 [64].

    Returns int8 [64, 512, 512, 4].
    """
    from concourse.bass_utils import run_bass_kernel_spmd

    if "nc" not in _PROGRAM_CACHE:
        _PROGRAM_CACHE["nc"] = _build_program()
    nc = _PROGRAM_CACHE["nc"]

    F = x.shape[0]
    assert F == N_CORES * F_PER_CORE

    in_maps = _make_in_maps(x, w, b)
    res = run_bass_kernel_spmd(nc, in_maps, core_ids=list(range(N_CORES)))

    out = np.empty((F, H_OUT, 512, D), dtype=np.int8)
    for core in range(N_CORES):
        lo = core * F_PER_CORE
        y = res.results[core]["y"][:, :, :WD_OUT]  # [8, 512, 2048] int8 (unpadded)
        out[lo : lo + F_PER_CORE] = np.ascontiguousarray(y).reshape(
            F_PER_CORE, H_OUT, 512, D
        )
    return out
